# revision 9
# baseline (speedup 1.0000x reference)
"""Multi-head attention (B=4, S=2048, E=1024, H=16, D=64) on 8 TRN2 NeuronCores.

Sharding: tensor-parallel over heads -- core c computes heads 2c and 2c+1.
Each core receives the full x (cast bf16) plus its [E, 128] slices of
Wq/Wk/Wv and biases, and produces out[:, :, 128c:128c+128]; the host
concatenates along the feature dim.

Per-core dataflow (all transposes ride the DMA xbar, none on the PE):
  x  --DMA-transpose-->  xT [E-chunk=128, S] (bf16)
  qT = (Wq^T xT)/8 + bq/8   [128(d,2 heads), S]   (PE + DVE psum->sbuf)
  kT =  Wk^T xT + bk        [128, S]
  vT =  Wv^T xT  --DVE bf16--> vt [128, 512] --2 DMA-transposes-->
        v_sb [128, 4, 256] per jh (cols per tile: v_h0|ones|0s|v_h1|ones|0s)
  scoresT[sk, sq] = kT^T qT  (K=64 per head; the two heads' matmuls are
        row-tiled at base partitions 0/64 and run concurrently)
  exp:  ACT Exp(x + maskbias) -> ex bf16 [128, 1024]
  yT_aug += v_aug^T ex  (K=128, accumulated over the 16 key tiles; row 64
        of each head's 128-col weight block carries the softmax denom)
  yT_aug --DVE bf16--> y --1 DMA-transpose per head--> y_nat [128, 4, 96]
        --DVE recip+scale--> ob --one DMA--> out[b, 512j:512j+512, :]

The DMA xbar flattens a 3D transpose destination [128, nblk, w] in
natural order (logical row r lands at partition r % 128, block r // 128),
so one [64, 512] transpose per (jh, head) fills the four key tiles of
that jh block in place, and one [96, 512] transpose per (j, head) yields
naturally-ordered query chunks.

Emission is one global software pipeline over all 256 (b, j, i) tiles:
scores(n)+exp(n) at slot n, PV at slot n-3, the (b, j) normalize/store
tail and the next batch's projections dripped into the slots between.
The ScalarE exp chain (~1.14 us/tile) is the critical resource; PE, DVE,
and the Sync DMA queue are kept strictly below it.
"""

import os
import sys
import types

import numpy as np
import ml_dtypes

import concourse.bass as bass
import concourse.tile as tile
from concourse import bacc, mybir
from concourse.bass_utils import run_bass_kernel_spmd

B, S, E, H, D = 4, 2048, 1024, 16, 64
NCORES = 8
DHC = (H // NCORES) * D  # 128 feature cols per core (2 heads)
NEG = -1.0e9  # additive mask bias for masked-out keys
BF16 = mybir.dt.bfloat16
F32 = mybir.dt.float32
SK = S // 128  # 16 key tiles per batch
SQ = S // 512  # 4 query blocks per batch

# DMA-transpose 3D destination flatten order: True if logical row r maps
# to partition r // nblk, block r % nblk (block-minor); False if it maps
# to partition r % 128, block r // 128 (natural).  Measured on HW: the
# xbar writes natural order, so no key/query permutation is needed.
PMAJ = False

LAST_RESULTS = None  # BassKernelResults of the most recent kernel() call


def _install_trace_hook():
    """Register the axon NTFF-profile hook so BASS_TRACE=1 works.

    The concourse trace path imports antenv.axon_hooks, which this image
    doesn't ship; synthesize it and register the ctypes-based hook.
    """
    try:
        import antenv

        if "antenv.axon_hooks" in sys.modules:
            return
        mod = types.ModuleType("antenv.axon_hooks")
        _hook = [None]
        mod.set_axon_ntff_profile_hook = lambda h: _hook.__setitem__(0, h)
        mod.get_axon_ntff_profile_hook = lambda: _hook[0]
        sys.modules["antenv.axon_hooks"] = mod
        antenv.axon_hooks = mod
        from trn_agent_boot.trn_boot import _ntff_profile_via_ctypes

        so = "/opt/axon/libaxon_pjrt.so"
        if os.path.exists(so):
            mod.set_axon_ntff_profile_hook(_ntff_profile_via_ctypes(so))
    except Exception:
        pass


_install_trace_hook()


class _Ctx:
    """Shared emission state for one core's program."""


def _setup(nc, tc, ctx, aps, has_bv):
    s = _Ctx()
    (s.x, wq, wk, wv, bq, bk, bv, maskb, s.out) = aps
    s.has_bv = has_bv

    singles = ctx.enter_context(tc.tile_pool(name="singles", bufs=1))
    s.xt_pool = ctx.enter_context(tc.tile_pool(name="xt", bufs=16))
    s.qk_pool = ctx.enter_context(tc.tile_pool(name="qk", bufs=4))
    s.v_pool = ctx.enter_context(tc.tile_pool(name="v", bufs=2))
    s.vt_pool = ctx.enter_context(tc.tile_pool(name="vt", bufs=2))
    s.exp_pool = ctx.enter_context(tc.tile_pool(name="exp", bufs=8))
    s.y_pool = ctx.enter_context(tc.tile_pool(name="y", bufs=4))
    s.ynat_pool = ctx.enter_context(tc.tile_pool(name="ynat", bufs=2))
    s.out_pool = ctx.enter_context(tc.tile_pool(name="outs", bufs=2))
    s.rc_pool = ctx.enter_context(tc.tile_pool(name="rc", bufs=4))
    # PSUM budget (8 banks): scores 2x[128,1024]=4, PV accum 3x[128,512]=3,
    # projection accum 1x[128,512]=1.
    s.ps_pool = ctx.enter_context(tc.tile_pool(name="ps", bufs=2, space="PSUM"))
    s.py_pool = ctx.enter_context(tc.tile_pool(name="py", bufs=3, space="PSUM"))
    s.prj_pool = ctx.enter_context(tc.tile_pool(name="prj", bufs=1, space="PSUM"))

    # One HWDGE DMA for all weights, one for all small constants.
    wcat_sb = singles.tile([128, 3, 8, 128], BF16, tag="wcat")
    nc.sync.dma_start(out=wcat_sb[:, :, :, :], in_=wq)
    s.w_sb = {"wq": wcat_sb[:, 0], "wk": wcat_sb[:, 1], "wv": wcat_sb[:, 2]}
    consts_sb = singles.tile([128, 66], F32, tag="consts")
    nc.sync.dma_start(out=consts_sb[:, :], in_=bq)
    s.bq_sb = consts_sb[:, 0:1]
    s.bk_sb = consts_sb[:, 1:2]
    s.maskb = consts_sb  # bias for (b, i) at column 2 + 16*b + i
    if has_bv:
        s.bv_sb = singles.tile([128, DHC], F32, tag="bv")
        bv_bcast = bass.AP(tensor=bv.tensor, offset=bv.offset,
                           ap=[[0, 128]] + bv.ap[1:])
        nc.gpsimd.dma_start(out=s.bv_sb[:, :], in_=bv_bcast)
    # Warm the ACT exp table set while the first x slices are in flight,
    # so the first real softmax exp doesn't pay the ~2.7us table load.
    s.scratch = singles.tile([128, 1], F32, tag="scratch")
    nc.vector.memset(s.scratch[:, :], 0.0)
    nc.scalar.activation(
        out=s.scratch[:, :], in_=s.scratch[:, :],
        func=mybir.ActivationFunctionType.Exp, bias=0.0, scale=1.0)
    return s


def _x_load_b0(nc, s):
    """Batch-0 xT load: 512-row slices for jh0/jh1 (so the first
    projection blocks can start after ~8 us of dispatch), then one
    1024-row half per chunk for jh2/jh3."""
    xt = []
    for c in range(8):
        t = s.xt_pool.tile([128, S], BF16, tag="xt", name=f"xt0_{c}")
        xt.append(t)
    s.xt = {0: xt}
    for q in range(2):
        for c in range(8):
            nc.sync.dma_start_transpose(
                out=xt[c][:, 512 * q:512 * (q + 1)],
                in_=s.x[0, 512 * q:512 * (q + 1), 128 * c:128 * (c + 1)])
    for c in range(8):
        nc.sync.dma_start_transpose(
            out=xt[c][:, 1024:2048],
            in_=s.x[0, 1024:2048, 128 * c:128 * (c + 1)])


def _x_load(nc, s, b):
    """Dispatch batch b's full-chunk xT transposes (8 sync DMAs)."""
    xt = []
    for c in range(8):
        t = s.xt_pool.tile([128, S], BF16, tag="xt", name=f"xt{b}_{c}")
        xt.append(t)
    s.xt[b] = xt
    for c in range(8):
        nc.sync.dma_start_transpose(
            out=xt[c][:, :], in_=s.x[b, :, 128 * c:128 * (c + 1)])


def _kslice(s, kT, h, i):
    """Stationary kT slice for key tile i, matching the v permutation."""
    hp_lo = 64 * h
    if PMAJ:
        jh, t = i // 4, i % 4
        return kT[hp_lo:hp_lo + 64, 512 * jh + t:512 * (jh + 1):4]
    return kT[hp_lo:hp_lo + 64, 128 * i:128 * (i + 1)]


def _gen_proj(nc, s, b, split=False):
    """Generator: emits batch b's xT loads + q/k/v projections.

    Registers output tiles in s.proj[b] up front. Emission order is
    q[jh0], k[jh0], v[jh0], (yield "BOOT" when split), k1, v1, ...,
    q[1..3] -- so attention on the first key tiles can start as soon as
    the first jh block of projections has run.
    """
    mult, add = mybir.AluOpType.mult, mybir.AluOpType.add

    qT = s.qk_pool.tile([128, S], BF16, tag="qk", name=f"qT{b}")
    kT = s.qk_pool.tile([128, S], BF16, tag="qk", name=f"kT{b}")
    v_sb = s.v_pool.tile([128, SK, 256], BF16, tag="v", name=f"v{b}")
    s.proj = getattr(s, "proj", {})
    s.proj[b] = (qT, kT, v_sb)

    if split:
        xt = s.xt[0]
    else:
        xt = []
        for c in range(8):
            t = s.xt_pool.tile([128, S], BF16, tag="xt", name=f"xt{b}_{c}")
            xt.append(t)
        for c in range(8):
            nc.sync.dma_start_transpose(
                out=xt[c][:, :], in_=s.x[b, :, 128 * c:128 * (c + 1)])
            if c % 2 == 1:
                yield "c"
    for h in range(2):
        nc.vector.memset(v_sb[:, :, 128 * h + 64:128 * (h + 1)], 0.0)
        nc.vector.memset(v_sb[:, :, 128 * h + 64:128 * h + 65], 1.0)
    yield "c"

    ngroups = [0]

    def accum_tile(name):
        # During the batch-0 bootstrap the scores pool is still idle, so
        # alternate with it for a 2-deep projection pipeline (the single
        # prj bank would serialize each group behind the previous group's
        # psum->sbuf copy).
        ngroups[0] += 1
        if split and ngroups[0] == 2:
            return s.ps_pool.tile([128, 512], F32, tag="ps", name=name)
        return s.prj_pool.tile([128, 512], F32, tag="prj", name=name)

    def q_or_k(name, dest, bias_sb, scale, jhs):
        w = s.w_sb[name]
        for jh in jhs:
            ps = accum_tile("pj")
            for c in range(8):
                nc.tensor.matmul(
                    ps[:, :], w[:, c, :], xt[c][:, 512 * jh:512 * (jh + 1)],
                    start=(c == 0), stop=(c == 7))
                if c % 2 == 1:
                    yield "c"
            nc.vector.tensor_scalar(
                out=dest[:, 512 * jh:512 * (jh + 1)], in0=ps[:, :],
                scalar1=scale, scalar2=bias_sb[:, :], op0=mult, op1=add)
            yield "c"

    def v_proj(jh):
        # v: project to vT, DVE-cast to bf16, then one DMA-transpose per
        # head back to natural [s, d] layout beside the ones columns.
        w = s.w_sb["wv"]
        ps = accum_tile("pv")
        for c in range(8):
            nc.tensor.matmul(
                ps[:, :], w[:, c, :], xt[c][:, 512 * jh:512 * (jh + 1)],
                start=(c == 0), stop=(c == 7))
            if c % 2 == 1:
                yield "c"
        vt = s.vt_pool.tile([128, 512], BF16, tag="vt", name="vt")
        nc.vector.tensor_copy(out=vt[:, :], in_=ps[:, :])
        yield "c"
        for h in range(2):
            nc.sync.dma_start_transpose(
                out=v_sb[:, 4 * jh:4 * (jh + 1), 128 * h:128 * h + 64],
                in_=vt[64 * h:64 * (h + 1), :])
        yield "c"

    yield from q_or_k("wq", qT, s.bq_sb, 0.125, [0])
    yield from q_or_k("wk", kT, s.bk_sb, 1.0, [0])
    yield from v_proj(0)
    if split:
        yield "BOOT"
    for jh in range(1, 4):
        yield from q_or_k("wk", kT, s.bk_sb, 1.0, [jh])
        yield from v_proj(jh)
    yield from q_or_k("wq", qT, s.bq_sb, 0.125, [1, 2, 3])


def _emit_scores_exp(nc, s, b, j, i):
    qT, kT, v_sb = s.proj[b]
    jsl = slice(512 * j, 512 * (j + 1))
    ps = s.ps_pool.tile([128, 1024], F32, tag="ps", name="psc")
    for h in range(2):
        hp = slice(64 * h, 64 * (h + 1))
        nc.tensor.matmul(
            ps[:, 512 * h:512 * (h + 1)],
            _kslice(s, kT, h, i), qT[hp, jsl],
            start=True, stop=True)
    ex = s.exp_pool.tile([128, 1024], BF16, tag="exp", name="ex")
    nc.scalar.activation(
        out=ex[:, :], in_=ps[:, :],
        func=mybir.ActivationFunctionType.Exp,
        bias=s.maskb[:, 2 + 16 * b + i:3 + 16 * b + i], scale=1.0)
    return ex


def _emit_pv(nc, s, b, j, i, ex):
    _, _, v_sb = s.proj[b]
    s.py = getattr(s, "py", {})
    if (b, j) not in s.py:
        s.py[(b, j)] = [
            s.py_pool.tile([128, 512], F32, tag="py", name=f"py{h}")
            for h in range(2)]
    py = s.py[(b, j)]
    for h in range(2):
        nc.tensor.matmul(
            py[h][:, :], v_sb[:, i, 128 * h:128 * (h + 1)],
            ex[:, 512 * h:512 * (h + 1)],
            start=(i == 0), stop=(i == SK - 1))


def _gen_tail(nc, s, b, j):
    """Normalize + transpose back + store for one (b, j) block.

    py rows (both heads): 0-63 y values, 64 denom, 65-127 junk/zeros.
    The DVE casts rows 0-95 to bf16 (rows 65-95 transpose into ignored
    columns), one DMA xbar transpose per head flips to [query, feature]
    (query order permuted as q = 4p + t), and the DVE normalizes into a
    [128, 4, 128] f32 block stored with a single DMA whose DRAM access
    pattern undoes the permutation.
    """
    mult = mybir.AluOpType.mult
    py = s.py.pop((b, j))
    ys = []
    for h in range(2):
        y = s.y_pool.tile([128, 512], BF16, tag="y", name=f"y{h}")
        nc.vector.tensor_copy(out=y[0:96, :], in_=py[h][0:96, :])
        ys.append(y)
    yield "t"
    y_nat = s.ynat_pool.tile([128, 8, 96], BF16, tag="ynat", name="ynat")
    for h in range(2):
        nc.sync.dma_start_transpose(
            out=y_nat[:, 4 * h:4 * (h + 1), :], in_=ys[h][0:96, :])
    yield "t"
    ob = s.out_pool.tile([128, 4, DHC], F32, tag="outs", name="ob")
    for t in range(4):
        for h in range(2):
            rc = s.rc_pool.tile([128, 1], F32, tag="rc", name="rc")
            nc.vector.reciprocal(rc[:, :], y_nat[:, 4 * h + t, 64:65])
            nc.vector.tensor_scalar(
                out=ob[:, t, 64 * h:64 * (h + 1)],
                in0=y_nat[:, 4 * h + t, 0:64],
                scalar1=rc[:, :], scalar2=None, op0=mult)
        if s.has_bv:
            nc.vector.tensor_add(ob[:, t, :], ob[:, t, :], s.bv_sb[:, :])
        yield "t"
    pat = "(p t) d -> p t d" if PMAJ else "(t p) d -> p t d"
    dst = s.out[b, 512 * j:512 * (j + 1), :].rearrange(pat, t=4)
    nc.sync.dma_start(out=dst, in_=ob[:, :, :])
    yield "t"


def _emit_body(nc, tc, ctx, aps, has_bv):
    from collections import deque

    s = _setup(nc, tc, ctx, aps, has_bv)
    _x_load_b0(nc, s)
    # Tile derives dependencies from emission order, so proj work must be
    # emitted before the attention instructions that read it; the drip
    # pacing below keeps the PE queue fed without letting projection
    # matmuls that wait on not-yet-landed xT slices block the queue.
    pend = deque()
    tails = deque()
    gp0 = _gen_proj(nc, s, 0, split=True)
    for tok in gp0:
        if tok == "BOOT":
            break
    pend.append(gp0)

    DONE = object()

    def drip(n):
        for _ in range(n):
            if tails:
                if next(tails[0], DONE) is DONE:
                    tails.popleft()
                continue
            if pend:
                if next(pend[0], DONE) is DONE:
                    pend.popleft()

    slots = [(b, j, i) for b in range(B) for j in range(SQ) for i in range(SK)]
    lag = deque()
    for n, (b, j, i) in enumerate(slots):
        if (j, i) == (0, 0) and b + 1 < B:
            pend.append(_gen_proj(nc, s, b + 1))
        ex = _emit_scores_exp(nc, s, b, j, i)
        lag.append((b, j, i, ex))
        if n >= 3:
            bb, jj, ii, exx = lag.popleft()
            _emit_pv(nc, s, bb, jj, ii, exx)
            if ii == SK - 1:
                tails.append(_gen_tail(nc, s, bb, jj))
        drip(5 if n < 24 else 2)
    while lag:
        bb, jj, ii, exx = lag.popleft()
        _emit_pv(nc, s, bb, jj, ii, exx)
        if ii == SK - 1:
            tails.append(_gen_tail(nc, s, bb, jj))
    while tails or pend:
        drip(1)


def _build(has_bv):
    from contextlib import ExitStack

    nc = bacc.Bacc("TRN2", target_bir_lowering=False, debug=False)
    x = nc.dram_tensor("x", [B, S, E], BF16, kind="ExternalInput").ap()
    wq = nc.dram_tensor("wcat", [128, 3, 8, 128], BF16,
                        kind="ExternalInput").ap()
    bq = nc.dram_tensor("consts", [128, 66], F32, kind="ExternalInput").ap()
    bv = nc.dram_tensor("bv", [1, DHC], F32, kind="ExternalInput").ap()
    out = nc.dram_tensor("out", [B, S, DHC], F32, kind="ExternalOutput").ap()
    aps = (x, wq, None, None, bq, None, bv, None, out)
    with tile.TileContext(nc) as tc:
        with ExitStack() as ctx:
            _emit_body(nc, tc, ctx, aps, has_bv)
    nc.compile()
    return nc


_BUILD_CACHE = {}


def _get_built(has_bv):
    if has_bv not in _BUILD_CACHE:
        _BUILD_CACHE[has_bv] = _build(has_bv)
    return _BUILD_CACHE[has_bv]


def kernel(x, mask, Wq, bq, Wk, bk, Wv, bv):
    global LAST_RESULTS
    bf16 = ml_dtypes.bfloat16
    x_bf = np.asarray(x, dtype=np.float32).astype(bf16)
    mask_f = np.asarray(mask).astype(np.float32)
    maskb = (mask_f - 1.0) * (-NEG)  # 0 where mask==1, NEG where mask==0
    if PMAJ:
        # col (b, i=4*jh+t)[p] = bias for key s = 512*jh + 4*p + t
        maskb = np.ascontiguousarray(
            maskb.reshape(B, 4, 128, 4).transpose(2, 0, 1, 3)
            .reshape(128, B, 16)).astype(np.float32)
    else:
        maskb = np.ascontiguousarray(
            maskb.reshape(B, S // 128, 128).transpose(2, 0, 1)
        ).astype(np.float32)

    has_bv = bool(np.any(np.asarray(bv) != 0))
    nc = _get_built(has_bv)

    in_maps = []
    for c in range(NCORES):
        sl = slice(DHC * c, DHC * (c + 1))

        def warr(w):
            w = np.asarray(w, dtype=np.float32)[:, sl].astype(bf16)
            return np.ascontiguousarray(
                w.reshape(8, 128, 128).transpose(1, 0, 2))

        wcat = np.stack([warr(Wq), warr(Wk), warr(Wv)], axis=1)
        consts = np.empty((128, 66), dtype=np.float32)
        consts[:, 0] = np.asarray(bq, dtype=np.float32)[sl] / 8.0
        consts[:, 1] = np.asarray(bk, dtype=np.float32)[sl]
        consts[:, 2:66] = maskb.reshape(128, 64)
        in_maps.append({
            "x": x_bf,
            "wcat": np.ascontiguousarray(wcat),
            "consts": consts,
            "bv": np.ascontiguousarray(
                np.asarray(bv, dtype=np.float32)[sl].reshape(1, DHC)),
        })

    res = run_bass_kernel_spmd(nc, in_maps, core_ids=list(range(NCORES)))
    LAST_RESULTS = res
    return np.concatenate([res.results[c]["out"] for c in range(NCORES)],
                          axis=-1)


# revision 16
# speedup vs baseline: 1.0377x; 1.0377x over previous
"""Multi-head attention (B=4, S=2048, E=1024, H=16, D=64) on 8 TRN2 NeuronCores.

Sharding: tensor-parallel over heads -- core c computes heads 2c and 2c+1.
Each core receives the full x (cast bf16) plus its [E, 128] slices of
Wq/Wk/Wv and biases, and produces out[:, :, 128c:128c+128]; the host
concatenates along the feature dim.

Per-core dataflow (all transposes ride the DMA xbar, none on the PE):
  x  --DMA-transpose-->  xT [E-chunk=128, S] (bf16)
  qT = (Wq^T xT)/8 + bq/8   [128(d,2 heads), S]   (PE + DVE psum->sbuf)
  kT =  Wk^T xT + bk        [128, S]
  vT =  Wv^T xT  --DVE bf16--> vt [128, 512] --2 DMA-transposes-->
        v_sb [128, 4, 256] per jh (cols per tile: v_h0|ones|0s|v_h1|ones|0s)
  scoresT[sk, sq] = kT^T qT  (K=64 per head; the two heads' matmuls are
        row-tiled at base partitions 0/64 and run concurrently)
  exp:  ACT Exp(x + maskbias) -> ex bf16 [128, 1024]
  yT_aug += v_aug^T ex  (K=128, accumulated over the 16 key tiles; row 64
        of each head's 128-col weight block carries the softmax denom)
  yT_aug --DVE bf16--> y --1 DMA-transpose per head--> y_nat [128, 4, 96]
        --DVE recip+scale--> ob --one DMA--> out[b, 512j:512j+512, :]

The DMA xbar flattens a 3D transpose destination [128, nblk, w] in
natural order (logical row r lands at partition r % 128, block r // 128),
so one [64, 512] transpose per (jh, head) fills the four key tiles of
that jh block in place, and one [96, 512] transpose per (j, head) yields
naturally-ordered query chunks.

Emission is one global software pipeline over all 256 (b, j, i) tiles:
scores(n)+exp(n) at slot n, PV at slot n-3, the (b, j) normalize/store
tail and the next batch's projections dripped into the slots between.
The ScalarE exp chain (~1.14 us/tile) is the critical resource; PE, DVE,
and the Sync DMA queue are kept strictly below it.
"""

import os
import sys
import types

import numpy as np
import ml_dtypes

import concourse.bass as bass
import concourse.tile as tile
from concourse import bacc, mybir
from concourse.bass_utils import run_bass_kernel_spmd

B, S, E, H, D = 4, 2048, 1024, 16, 64
NCORES = 8
DHC = (H // NCORES) * D  # 128 feature cols per core (2 heads)
NEG = -1.0e9  # additive mask bias for masked-out keys
BF16 = mybir.dt.bfloat16
F32 = mybir.dt.float32
SK = S // 128  # 16 key tiles per batch
SQ = S // 512  # 4 query blocks per batch

# DMA-transpose 3D destination flatten order: True if logical row r maps
# to partition r // nblk, block r % nblk (block-minor); False if it maps
# to partition r % 128, block r // 128 (natural).  Measured on HW: the
# xbar writes natural order, so no key/query permutation is needed.
PMAJ = False

LAST_RESULTS = None  # BassKernelResults of the most recent kernel() call


def _install_trace_hook():
    """Register the axon NTFF-profile hook so BASS_TRACE=1 works.

    The concourse trace path imports antenv.axon_hooks, which this image
    doesn't ship; synthesize it and register the ctypes-based hook.
    """
    try:
        import antenv

        if "antenv.axon_hooks" in sys.modules:
            return
        mod = types.ModuleType("antenv.axon_hooks")
        _hook = [None]
        mod.set_axon_ntff_profile_hook = lambda h: _hook.__setitem__(0, h)
        mod.get_axon_ntff_profile_hook = lambda: _hook[0]
        sys.modules["antenv.axon_hooks"] = mod
        antenv.axon_hooks = mod
        from trn_agent_boot.trn_boot import _ntff_profile_via_ctypes

        so = "/opt/axon/libaxon_pjrt.so"
        if os.path.exists(so):
            mod.set_axon_ntff_profile_hook(_ntff_profile_via_ctypes(so))
    except Exception:
        pass


_install_trace_hook()


class _Ctx:
    """Shared emission state for one core's program."""


def _setup(nc, tc, ctx, aps, has_bv):
    s = _Ctx()
    (s.x, wq, wk, wv, bq, bk, bv, maskb, s.out) = aps
    s.has_bv = has_bv

    singles = ctx.enter_context(tc.tile_pool(name="singles", bufs=1))
    s.xt_pool = ctx.enter_context(tc.tile_pool(name="xt", bufs=8))
    s.qk_pool = ctx.enter_context(tc.tile_pool(name="qk", bufs=4))
    s.v_pool = ctx.enter_context(tc.tile_pool(name="v", bufs=2))
    s.vt_pool = ctx.enter_context(tc.tile_pool(name="vt", bufs=2))
    s.exp_pool = ctx.enter_context(tc.tile_pool(name="exp", bufs=8))
    s.y_pool = ctx.enter_context(tc.tile_pool(name="y", bufs=4))
    s.ynat_pool = ctx.enter_context(tc.tile_pool(name="ynat", bufs=2))
    s.out_pool = ctx.enter_context(tc.tile_pool(name="outs", bufs=2))
    s.rc_pool = ctx.enter_context(tc.tile_pool(name="rc", bufs=4))
    # PSUM budget (8 banks): scores 2x[128,1024]=4, PV accum 3x[128,512]=3,
    # projection accum 1x[128,512]=1.
    s.ps_pool = ctx.enter_context(tc.tile_pool(name="ps", bufs=2, space="PSUM"))
    s.py_pool = ctx.enter_context(tc.tile_pool(name="py", bufs=3, space="PSUM"))
    s.prj_pool = ctx.enter_context(tc.tile_pool(name="prj", bufs=1, space="PSUM"))

    # One HWDGE DMA for all weights, one for all small constants -- on the
    # scalar queue so the sync queue runs x transposes from t=0 without
    # same-queue throttling behind these transfers.
    wcat_sb = singles.tile([128, 3, 8, 128], BF16, tag="wcat")
    nc.scalar.dma_start(out=wcat_sb[:, :, :, :], in_=wq)
    s.w_sb = {"wq": wcat_sb[:, 0], "wk": wcat_sb[:, 1], "wv": wcat_sb[:, 2]}
    consts_sb = singles.tile([128, 66], F32, tag="consts")
    nc.scalar.dma_start(out=consts_sb[:, :], in_=bq)
    s.bq_sb = consts_sb[:, 0:1]
    s.bk_sb = consts_sb[:, 1:2]
    s.maskb = consts_sb  # bias for (b, i) at column 2 + 16*b + i
    if has_bv:
        s.bv_sb = singles.tile([128, DHC], F32, tag="bv")
        bv_bcast = bass.AP(tensor=bv.tensor, offset=bv.offset,
                           ap=[[0, 128]] + bv.ap[1:])
        nc.gpsimd.dma_start(out=s.bv_sb[:, :], in_=bv_bcast)
    # Warm the ACT exp table set while the first x slices are in flight,
    # so the first real softmax exp doesn't pay the ~2.7us table load.
    s.scratch = singles.tile([128, 1], F32, tag="scratch")
    nc.scalar.activation(
        out=s.scratch[:, :], in_=consts_sb[:, 0:1],
        func=mybir.ActivationFunctionType.Exp, bias=consts_sb[:, 1:2],
        scale=1.0)
    return s


def _x_alloc(s, b):
    """xT tiles for batch b: 4 pair-tiles [128, 2, S] (E-chunks 2p, 2p+1).

    The xbar writes a 3D transpose destination in natural order, so one
    [rows, 256] transpose fills both chunks of a pair -- halving the
    ~1 us fixed dispatch cost per DMA on the sync queue.
    """
    xt = [s.xt_pool.tile([128, 2, S], BF16, tag="xt", name=f"xt{b}_{p}")
          for p in range(4)]
    s.xt = getattr(s, "xt", {})
    s.xt[b] = xt
    return xt


def _x_load_b0(nc, s):
    """Batch-0 xT load: 512-row slices for jh0 then jh1 (so the first
    projection blocks can start after ~6 us of dispatch), then one
    1024-row half per pair for jh2/jh3."""
    xt = _x_alloc(s, 0)
    for q in range(2):
        for p in range(4):
            nc.sync.dma_start_transpose(
                out=xt[p][:, :, 512 * q:512 * (q + 1)],
                in_=s.x[0, 512 * q:512 * (q + 1), 256 * p:256 * (p + 1)])
    for p in range(4):
        nc.sync.dma_start_transpose(
            out=xt[p][:, :, 1024:2048],
            in_=s.x[0, 1024:2048, 256 * p:256 * (p + 1)])


def _x_load(nc, s, b):
    """Dispatch batch b's xT transposes (4 pair-chunk sync DMAs)."""
    xt = _x_alloc(s, b)
    for p in range(4):
        nc.sync.dma_start_transpose(
            out=xt[p][:, :, :], in_=s.x[b, :, 256 * p:256 * (p + 1)])


def _kslice(s, kT, h, i):
    """Stationary kT slice for key tile i, matching the v permutation."""
    hp_lo = 64 * h
    if PMAJ:
        jh, t = i // 4, i % 4
        return kT[hp_lo:hp_lo + 64, 512 * jh + t:512 * (jh + 1):4]
    return kT[hp_lo:hp_lo + 64, 128 * i:128 * (i + 1)]


def _gen_proj(nc, s, b, split=False):
    """Generator: emits batch b's xT loads + q/k/v projections.

    Registers output tiles in s.proj[b] up front. Emission order is
    q[jh0], k[jh0], v[jh0], (yield "BOOT" when split), k1, v1, ...,
    q[1..3] -- so attention on the first key tiles can start as soon as
    the first jh block of projections has run.
    """
    mult, add = mybir.AluOpType.mult, mybir.AluOpType.add

    qT = s.qk_pool.tile([128, S], BF16, tag="qk", name=f"qT{b}")
    kT = s.qk_pool.tile([128, S], BF16, tag="qk", name=f"kT{b}")
    v_sb = s.v_pool.tile([128, SK, 256], BF16, tag="v", name=f"v{b}")
    s.proj = getattr(s, "proj", {})
    s.proj[b] = (qT, kT, v_sb)

    xt = s.xt[b]
    for h in range(2):
        nc.vector.memset(v_sb[:, :, 128 * h + 64:128 * (h + 1)], 0.0)
        nc.vector.memset(v_sb[:, :, 128 * h + 64:128 * h + 65], 1.0)
    yield "c"

    ngroups = [0]

    def accum_tile(name):
        # During the batch-0 bootstrap the scores pool is still idle, so
        # alternate with it for a 2-deep projection pipeline (the single
        # prj bank would serialize each group behind the previous group's
        # psum->sbuf copy).
        ngroups[0] += 1
        if split and ngroups[0] == 2:
            return s.ps_pool.tile([128, 512], F32, tag="ps", name=name)
        return s.prj_pool.tile([128, 512], F32, tag="prj", name=name)

    def q_or_k(name, dest, bias_sb, scale, jhs):
        w = s.w_sb[name]
        for jh in jhs:
            ps = accum_tile("pj")
            for c in range(8):
                nc.tensor.matmul(
                    ps[:, :], w[:, c, :],
                    xt[c // 2][:, c % 2, 512 * jh:512 * (jh + 1)],
                    start=(c == 0), stop=(c == 7))
                if c % 2 == 1:
                    yield "c"
            nc.vector.tensor_scalar(
                out=dest[:, 512 * jh:512 * (jh + 1)], in0=ps[:, :],
                scalar1=scale, scalar2=bias_sb[:, :], op0=mult, op1=add)
            yield "c"

    def v_proj(jh):
        # v: project to vT, DVE-cast to bf16, then one DMA-transpose per
        # head back to natural [s, d] layout beside the ones columns.
        w = s.w_sb["wv"]
        ps = accum_tile("pv")
        for c in range(8):
            nc.tensor.matmul(
                ps[:, :], w[:, c, :],
                xt[c // 2][:, c % 2, 512 * jh:512 * (jh + 1)],
                start=(c == 0), stop=(c == 7))
            if c % 2 == 1:
                yield "c"
        vt = s.vt_pool.tile([128, 512], BF16, tag="vt", name="vt")
        nc.vector.tensor_copy(out=vt[:, :], in_=ps[:, :])
        yield "c"
        for h in range(2):
            nc.sync.dma_start_transpose(
                out=v_sb[:, 4 * jh:4 * (jh + 1), 128 * h:128 * h + 64],
                in_=vt[64 * h:64 * (h + 1), :])
        yield "c"

    yield from q_or_k("wq", qT, s.bq_sb, 0.125, [0])
    yield from q_or_k("wk", kT, s.bk_sb, 1.0, [0])
    yield from v_proj(0)
    if split:
        yield "BOOT"
    for jh in range(1, 4):
        yield from q_or_k("wk", kT, s.bk_sb, 1.0, [jh])
        yield from v_proj(jh)
    yield from q_or_k("wq", qT, s.bq_sb, 0.125, [1, 2, 3])


def _emit_scores_exp(nc, s, b, j, i):
    qT, kT, v_sb = s.proj[b]
    jsl = slice(512 * j, 512 * (j + 1))
    ps = s.ps_pool.tile([128, 1024], F32, tag="ps", name="psc")
    for h in range(2):
        hp = slice(64 * h, 64 * (h + 1))
        nc.tensor.matmul(
            ps[:, 512 * h:512 * (h + 1)],
            _kslice(s, kT, h, i), qT[hp, jsl],
            start=True, stop=True)
    ex = s.exp_pool.tile([128, 1024], BF16, tag="exp", name="ex")
    nc.scalar.activation(
        out=ex[:, :], in_=ps[:, :],
        func=mybir.ActivationFunctionType.Exp,
        bias=s.maskb[:, 2 + 16 * b + i:3 + 16 * b + i], scale=1.0)
    return ex


def _emit_pv(nc, s, b, j, i, ex):
    _, _, v_sb = s.proj[b]
    s.py = getattr(s, "py", {})
    if (b, j) not in s.py:
        s.py[(b, j)] = [
            s.py_pool.tile([128, 512], F32, tag="py", name=f"py{h}")
            for h in range(2)]
    py = s.py[(b, j)]
    for h in range(2):
        nc.tensor.matmul(
            py[h][:, :], v_sb[:, i, 128 * h:128 * (h + 1)],
            ex[:, 512 * h:512 * (h + 1)],
            start=(i == 0), stop=(i == SK - 1))


def _gen_tail(nc, s, b, j):
    """Normalize + transpose back + store for one (b, j) block.

    py rows (both heads): 0-63 y values, 64 denom, 65-127 junk/zeros.
    The DVE casts rows 0-95 to bf16 (rows 65-95 transpose into ignored
    columns), one DMA xbar transpose per head flips to [query, feature]
    (query order permuted as q = 4p + t), and the DVE normalizes into a
    [128, 4, 128] f32 block stored with a single DMA whose DRAM access
    pattern undoes the permutation.
    """
    mult = mybir.AluOpType.mult
    py = s.py.pop((b, j))
    ys = []
    for h in range(2):
        y = s.y_pool.tile([128, 512], BF16, tag="y", name=f"y{h}")
        nc.vector.tensor_copy(out=y[0:96, :], in_=py[h][0:96, :])
        ys.append(y)
    yield "t"
    y_nat = s.ynat_pool.tile([128, 8, 96], BF16, tag="ynat", name="ynat")
    for h in range(2):
        nc.sync.dma_start_transpose(
            out=y_nat[:, 4 * h:4 * (h + 1), :], in_=ys[h][0:96, :])
    yield "t"
    ob = s.out_pool.tile([128, 4, DHC], F32, tag="outs", name="ob")
    for t in range(4):
        for h in range(2):
            rc = s.rc_pool.tile([128, 1], F32, tag="rc", name="rc")
            nc.vector.reciprocal(rc[:, :], y_nat[:, 4 * h + t, 64:65])
            nc.vector.tensor_scalar(
                out=ob[:, t, 64 * h:64 * (h + 1)],
                in0=y_nat[:, 4 * h + t, 0:64],
                scalar1=rc[:, :], scalar2=None, op0=mult)
        if s.has_bv:
            nc.vector.tensor_add(ob[:, t, :], ob[:, t, :], s.bv_sb[:, :])
        yield "t"
    pat = "(p t) d -> p t d" if PMAJ else "(t p) d -> p t d"
    dst = s.out[b, 512 * j:512 * (j + 1), :].rearrange(pat, t=4)
    nc.sync.dma_start(out=dst, in_=ob[:, :, :])
    yield "t"


def _emit_body(nc, tc, ctx, aps, has_bv):
    from collections import deque

    s = _setup(nc, tc, ctx, aps, has_bv)
    _x_load_b0(nc, s)
    # Tile derives dependencies from emission order, so proj work must be
    # emitted before the attention instructions that read it; the drip
    # pacing below keeps the PE queue fed without letting projection
    # matmuls that wait on not-yet-landed xT slices block the queue.
    pend = deque()
    tails = deque()
    gp0 = _gen_proj(nc, s, 0, split=True)
    for tok in gp0:
        if tok == "BOOT":
            break
    pend.append(gp0)

    DONE = object()

    def drip(n):
        for _ in range(n):
            if tails:
                if next(tails[0], DONE) is DONE:
                    tails.popleft()
                continue
            if pend:
                if next(pend[0], DONE) is DONE:
                    pend.popleft()

    slots = [(b, j, i) for b in range(B) for j in range(SQ) for i in range(SK)]
    lag = deque()
    for n, (b, j, i) in enumerate(slots):
        # Next batch's xT chunk dispatches go out inline (sync queue only,
        # no PE instructions) once this batch's own v/y DMA traffic has
        # been emitted; the projection matmuls are dripped only from slot
        # 24 on, by which time the xT transfers have landed -- an earlier
        # emission would block the in-order PE queue on the transfer.
        if b + 1 < B:
            m = 16 * j + i  # slot within batch b
            if m == (12 if b == 0 else 6):
                _x_load(nc, s, b + 1)
            if m == (30 if b == 0 else 24):
                pend.append(_gen_proj(nc, s, b + 1))
        ex = _emit_scores_exp(nc, s, b, j, i)
        lag.append((b, j, i, ex))
        if n >= 3:
            bb, jj, ii, exx = lag.popleft()
            _emit_pv(nc, s, bb, jj, ii, exx)
            if ii == SK - 1:
                tails.append(_gen_tail(nc, s, bb, jj))
        drip(5 if n < 24 else 2)
    while lag:
        bb, jj, ii, exx = lag.popleft()
        _emit_pv(nc, s, bb, jj, ii, exx)
        if ii == SK - 1:
            tails.append(_gen_tail(nc, s, bb, jj))
    while tails or pend:
        drip(1)


def _build(has_bv):
    from contextlib import ExitStack

    nc = bacc.Bacc("TRN2", target_bir_lowering=False, debug=False)
    x = nc.dram_tensor("x", [B, S, E], BF16, kind="ExternalInput").ap()
    wq = nc.dram_tensor("wcat", [128, 3, 8, 128], BF16,
                        kind="ExternalInput").ap()
    bq = nc.dram_tensor("consts", [128, 66], F32, kind="ExternalInput").ap()
    bv = nc.dram_tensor("bv", [1, DHC], F32, kind="ExternalInput").ap()
    out = nc.dram_tensor("out", [B, S, DHC], F32, kind="ExternalOutput").ap()
    aps = (x, wq, None, None, bq, None, bv, None, out)
    with tile.TileContext(nc) as tc:
        with ExitStack() as ctx:
            _emit_body(nc, tc, ctx, aps, has_bv)
    nc.compile()
    return nc


_BUILD_CACHE = {}


def _get_built(has_bv):
    if has_bv not in _BUILD_CACHE:
        _BUILD_CACHE[has_bv] = _build(has_bv)
    return _BUILD_CACHE[has_bv]


def kernel(x, mask, Wq, bq, Wk, bk, Wv, bv):
    global LAST_RESULTS
    bf16 = ml_dtypes.bfloat16
    x_bf = np.asarray(x, dtype=np.float32).astype(bf16)
    mask_f = np.asarray(mask).astype(np.float32)
    maskb = (mask_f - 1.0) * (-NEG)  # 0 where mask==1, NEG where mask==0
    if PMAJ:
        # col (b, i=4*jh+t)[p] = bias for key s = 512*jh + 4*p + t
        maskb = np.ascontiguousarray(
            maskb.reshape(B, 4, 128, 4).transpose(2, 0, 1, 3)
            .reshape(128, B, 16)).astype(np.float32)
    else:
        maskb = np.ascontiguousarray(
            maskb.reshape(B, S // 128, 128).transpose(2, 0, 1)
        ).astype(np.float32)

    has_bv = bool(np.any(np.asarray(bv) != 0))
    nc = _get_built(has_bv)

    in_maps = []
    for c in range(NCORES):
        sl = slice(DHC * c, DHC * (c + 1))

        def warr(w):
            w = np.asarray(w, dtype=np.float32)[:, sl].astype(bf16)
            return np.ascontiguousarray(
                w.reshape(8, 128, 128).transpose(1, 0, 2))

        wcat = np.stack([warr(Wq), warr(Wk), warr(Wv)], axis=1)
        consts = np.empty((128, 66), dtype=np.float32)
        consts[:, 0] = np.asarray(bq, dtype=np.float32)[sl] / 8.0
        consts[:, 1] = np.asarray(bk, dtype=np.float32)[sl]
        consts[:, 2:66] = maskb.reshape(128, 64)
        in_maps.append({
            "x": x_bf,
            "wcat": np.ascontiguousarray(wcat),
            "consts": consts,
            "bv": np.ascontiguousarray(
                np.asarray(bv, dtype=np.float32)[sl].reshape(1, DHC)),
        })

    res = run_bass_kernel_spmd(nc, in_maps, core_ids=list(range(NCORES)))
    LAST_RESULTS = res
    return np.concatenate([res.results[c]["out"] for c in range(NCORES)],
                          axis=-1)


# revision 21
# speedup vs baseline: 1.0465x; 1.0085x over previous
"""Multi-head attention (B=4, S=2048, E=1024, H=16, D=64) on 8 TRN2 NeuronCores.

Sharding: tensor-parallel over heads -- core c computes heads 2c and 2c+1.
Each core receives the full x (cast bf16) plus its [E, 128] slices of
Wq/Wk/Wv and biases, and produces out[:, :, 128c:128c+128]; the host
concatenates along the feature dim.

Per-core dataflow (all transposes ride the DMA xbar, none on the PE):
  x  --DMA-transpose-->  xT [E-chunk=128, S] (bf16)
  qT = (Wq^T xT)/8 + bq/8   [128(d,2 heads), S]   (PE + DVE psum->sbuf)
  kT =  Wk^T xT + bk        [128, S]
  vT =  Wv^T xT  --DVE bf16--> vt [128, 512] --2 DMA-transposes-->
        v_sb [128, 4, 256] per jh (cols per tile: v_h0|ones|0s|v_h1|ones|0s)
  scoresT[sk, sq] = kT^T qT  (K=64 per head; the two heads' matmuls are
        row-tiled at base partitions 0/64 and run concurrently)
  exp:  ACT Exp(x + maskbias) -> ex bf16 [128, 1024]
  yT_aug += v_aug^T ex  (K=128, accumulated over the 16 key tiles; row 64
        of each head's 128-col weight block carries the softmax denom)
  yT_aug --DVE bf16--> y --1 DMA-transpose per head--> y_nat [128, 4, 96]
        --DVE recip+scale--> ob --one DMA--> out[b, 512j:512j+512, :]

The DMA xbar flattens a 3D transpose destination [128, nblk, w] in
natural order (logical row r lands at partition r % 128, block r // 128),
so one [64, 512] transpose per (jh, head) fills the four key tiles of
that jh block in place, and one [96, 512] transpose per (j, head) yields
naturally-ordered query chunks.

Emission is one global software pipeline over all 256 (b, j, i) tiles:
scores(n)+exp(n) at slot n, PV at slot n-3, the (b, j) normalize/store
tail and the next batch's projections dripped into the slots between.
The ScalarE exp chain (~1.14 us/tile) is the critical resource; PE, DVE,
and the Sync DMA queue are kept strictly below it.
"""

import os
import sys
import types

import numpy as np
import ml_dtypes

import concourse.bass as bass
import concourse.tile as tile
from concourse import bacc, mybir
from concourse.bass_utils import run_bass_kernel_spmd

B, S, E, H, D = 4, 2048, 1024, 16, 64
NCORES = 8
DHC = (H // NCORES) * D  # 128 feature cols per core (2 heads)
NEG = -1.0e9  # additive mask bias for masked-out keys
BF16 = mybir.dt.bfloat16
F32 = mybir.dt.float32
SK = S // 128  # 16 key tiles per batch
SQ = S // 512  # 4 query blocks per batch

# DMA-transpose 3D destination flatten order: True if logical row r maps
# to partition r // nblk, block r % nblk (block-minor); False if it maps
# to partition r % 128, block r // 128 (natural).  Measured on HW: the
# xbar writes natural order, so no key/query permutation is needed.
PMAJ = False

LAST_RESULTS = None  # BassKernelResults of the most recent kernel() call


def _install_trace_hook():
    """Register the axon NTFF-profile hook so BASS_TRACE=1 works.

    The concourse trace path imports antenv.axon_hooks, which this image
    doesn't ship; synthesize it and register the ctypes-based hook.
    """
    try:
        import antenv

        if "antenv.axon_hooks" in sys.modules:
            return
        mod = types.ModuleType("antenv.axon_hooks")
        _hook = [None]
        mod.set_axon_ntff_profile_hook = lambda h: _hook.__setitem__(0, h)
        mod.get_axon_ntff_profile_hook = lambda: _hook[0]
        sys.modules["antenv.axon_hooks"] = mod
        antenv.axon_hooks = mod
        from trn_agent_boot.trn_boot import _ntff_profile_via_ctypes

        so = "/opt/axon/libaxon_pjrt.so"
        if os.path.exists(so):
            mod.set_axon_ntff_profile_hook(_ntff_profile_via_ctypes(so))
    except Exception:
        pass


_install_trace_hook()


class _Ctx:
    """Shared emission state for one core's program."""


def _setup(nc, tc, ctx, aps, has_bv):
    s = _Ctx()
    (s.x, wq, wk, wv, bq, bk, bv, maskb, s.out) = aps
    s.has_bv = has_bv

    singles = ctx.enter_context(tc.tile_pool(name="singles", bufs=1))
    s.xt_pool = ctx.enter_context(tc.tile_pool(name="xt", bufs=8))
    s.qk_pool = ctx.enter_context(tc.tile_pool(name="qk", bufs=4))
    s.v_pool = ctx.enter_context(tc.tile_pool(name="v", bufs=2))
    s.vt_pool = ctx.enter_context(tc.tile_pool(name="vt", bufs=2))
    s.exp_pool = ctx.enter_context(tc.tile_pool(name="exp", bufs=8))
    s.y_pool = ctx.enter_context(tc.tile_pool(name="y", bufs=4))
    s.ynat_pool = ctx.enter_context(tc.tile_pool(name="ynat", bufs=2))
    s.out_pool = ctx.enter_context(tc.tile_pool(name="outs", bufs=2))
    s.rc_pool = ctx.enter_context(tc.tile_pool(name="rc", bufs=4))
    # PSUM budget (8 banks): scores 2x[128,1024]=4, PV accum 3x[128,512]=3,
    # projection accum 1x[128,512]=1.
    s.ps_pool = ctx.enter_context(tc.tile_pool(name="ps", bufs=2, space="PSUM"))
    s.py_pool = ctx.enter_context(tc.tile_pool(name="py", bufs=3, space="PSUM"))
    s.prj_pool = ctx.enter_context(tc.tile_pool(name="prj", bufs=1, space="PSUM"))

    # All DMAs share a shallow dispatch ring, so the order here is the
    # bootstrap critical path: consts (tiny) first, then the weights are
    # loaded per-projection, interleaved just-in-time between the x slice
    # transposes by _x_load_b0 (which calls load_w).
    consts_sb = singles.tile([128, 66], F32, tag="consts")
    nc.scalar.dma_start(out=consts_sb[:, :], in_=bq)
    s.bq_sb = consts_sb[:, 0:1]
    s.bk_sb = consts_sb[:, 1:2]
    s.maskb = consts_sb  # bias for (b, i) at column 2 + 16*b + i
    wcat_sb = singles.tile([128, 3, 8, 128], BF16, tag="wcat")
    s.wq_dram = wq
    s.wcat_sb = wcat_sb
    s.w_sb = {"wq": wcat_sb[:, 0], "wk": wcat_sb[:, 1], "wv": wcat_sb[:, 2]}
    s.load_w = lambda k: nc.scalar.dma_start(
        out=wcat_sb[:, k, :, :], in_=s.wq_dram[:, k])
    if has_bv:
        s.bv_sb = singles.tile([128, DHC], F32, tag="bv")
        bv_bcast = bass.AP(tensor=bv.tensor, offset=bv.offset,
                           ap=[[0, 128]] + bv.ap[1:])
        nc.gpsimd.dma_start(out=s.bv_sb[:, :], in_=bv_bcast)
    s.scratch = singles.tile([128, 1], F32, tag="scratch")
    return s


def _warm_act(nc, s):
    # Warm the ACT exp table set while the first x slices are in flight,
    # so the first real softmax exp doesn't pay the ~2.7us table load.
    # Emitted after the weight DMA dispatches so it doesn't delay them on
    # the scalar queue.
    nc.scalar.activation(
        out=s.scratch[:, :], in_=s.maskb[:, 0:1],
        func=mybir.ActivationFunctionType.Exp, bias=s.maskb[:, 1:2],
        scale=1.0)


def _x_alloc(s, b):
    """xT tiles for batch b: 4 pair-tiles [128, 2, S] (E-chunks 2p, 2p+1).

    The xbar writes a 3D transpose destination in natural order, so one
    [rows, 256] transpose fills both chunks of a pair -- halving the
    ~1 us fixed dispatch cost per DMA on the sync queue.
    """
    xt = [s.xt_pool.tile([128, 2, S], BF16, tag="xt", name=f"xt{b}_{p}")
          for p in range(4)]
    s.xt = getattr(s, "xt", {})
    s.xt[b] = xt
    return xt


def _x_load_b0(nc, s):
    """Batch-0 xT load: 512-row slices for jh0 then jh1 (so the first
    projection blocks can start after ~6 us of dispatch), then one
    1024-row half per pair for jh2/jh3.  The per-projection weight DMAs
    are interleaved so each transfer lands just before its first use
    without pushing the x slices back in the shared dispatch ring."""
    xt = _x_alloc(s, 0)
    for q in range(2):
        for p in range(4):
            nc.sync.dma_start_transpose(
                out=xt[p][:, :, 512 * q:512 * (q + 1)],
                in_=s.x[0, 512 * q:512 * (q + 1), 256 * p:256 * (p + 1)])
        s.load_w(0 if q == 0 else 1)  # wq after jh0 slices, wk after jh1
    s.load_w(2)  # wv
    for p in range(4):
        nc.sync.dma_start_transpose(
            out=xt[p][:, :, 1024:2048],
            in_=s.x[0, 1024:2048, 256 * p:256 * (p + 1)])


def _x_load(nc, s, b):
    """Dispatch batch b's xT transposes (4 pair-chunk sync DMAs)."""
    xt = _x_alloc(s, b)
    for p in range(4):
        nc.sync.dma_start_transpose(
            out=xt[p][:, :, :], in_=s.x[b, :, 256 * p:256 * (p + 1)])


def _kslice(s, kT, h, i):
    """Stationary kT slice for key tile i, matching the v permutation."""
    hp_lo = 64 * h
    if PMAJ:
        jh, t = i // 4, i % 4
        return kT[hp_lo:hp_lo + 64, 512 * jh + t:512 * (jh + 1):4]
    return kT[hp_lo:hp_lo + 64, 128 * i:128 * (i + 1)]


def _gen_proj(nc, s, b, split=False):
    """Generator: emits batch b's xT loads + q/k/v projections.

    Registers output tiles in s.proj[b] up front. Emission order is
    q[jh0], k[jh0], v[jh0], (yield "BOOT" when split), k1, v1, ...,
    q[1..3] -- so attention on the first key tiles can start as soon as
    the first jh block of projections has run.
    """
    mult, add = mybir.AluOpType.mult, mybir.AluOpType.add

    qT = s.qk_pool.tile([128, S], BF16, tag="qk", name=f"qT{b}")
    kT = s.qk_pool.tile([128, S], BF16, tag="qk", name=f"kT{b}")
    v_sb = s.v_pool.tile([128, SK, 256], BF16, tag="v", name=f"v{b}")
    s.proj = getattr(s, "proj", {})
    s.proj[b] = (qT, kT, v_sb)

    xt = s.xt[b]
    for h in range(2):
        nc.vector.memset(v_sb[:, :, 128 * h + 64:128 * (h + 1)], 0.0)
        nc.vector.memset(v_sb[:, :, 128 * h + 64:128 * h + 65], 1.0)
    yield "c"

    ngroups = [0]

    def accum_tile(name):
        # During the batch-0 bootstrap the scores pool is still idle, so
        # alternate with it for a 2-deep projection pipeline (the single
        # prj bank would serialize each group behind the previous group's
        # psum->sbuf copy).
        ngroups[0] += 1
        if split and ngroups[0] == 2:
            return s.ps_pool.tile([128, 512], F32, tag="ps", name=name)
        return s.prj_pool.tile([128, 512], F32, tag="prj", name=name)

    def q_or_k(name, dest, bias_sb, scale, jhs):
        w = s.w_sb[name]
        for jh in jhs:
            ps = accum_tile("pj")
            for c in range(8):
                nc.tensor.matmul(
                    ps[:, :], w[:, c, :],
                    xt[c // 2][:, c % 2, 512 * jh:512 * (jh + 1)],
                    start=(c == 0), stop=(c == 7))
                if c % 2 == 1:
                    yield "c"
            nc.vector.tensor_scalar(
                out=dest[:, 512 * jh:512 * (jh + 1)], in0=ps[:, :],
                scalar1=scale, scalar2=bias_sb[:, :], op0=mult, op1=add)
            yield "c"

    def v_proj(jh):
        # v: project to vT, DVE-cast to bf16, then one DMA-transpose per
        # head back to natural [s, d] layout beside the ones columns.
        w = s.w_sb["wv"]
        ps = accum_tile("pv")
        for c in range(8):
            nc.tensor.matmul(
                ps[:, :], w[:, c, :],
                xt[c // 2][:, c % 2, 512 * jh:512 * (jh + 1)],
                start=(c == 0), stop=(c == 7))
            if c % 2 == 1:
                yield "c"
        vt = s.vt_pool.tile([128, 512], BF16, tag="vt", name="vt")
        nc.vector.tensor_copy(out=vt[:, :], in_=ps[:, :])
        yield "c"
        for h in range(2):
            nc.sync.dma_start_transpose(
                out=v_sb[:, 4 * jh:4 * (jh + 1), 128 * h:128 * h + 64],
                in_=vt[64 * h:64 * (h + 1), :])
        yield "c"

    yield from q_or_k("wq", qT, s.bq_sb, 0.125, [0])
    yield from q_or_k("wk", kT, s.bk_sb, 1.0, [0])
    yield from v_proj(0)
    if split:
        yield "BOOT"
    for jh in range(1, 4):
        yield from q_or_k("wk", kT, s.bk_sb, 1.0, [jh])
        yield from v_proj(jh)
    yield from q_or_k("wq", qT, s.bq_sb, 0.125, [1, 2, 3])


def _emit_scores_exp(nc, s, b, j, i):
    qT, kT, v_sb = s.proj[b]
    jsl = slice(512 * j, 512 * (j + 1))
    ps = s.ps_pool.tile([128, 1024], F32, tag="ps", name="psc")
    for h in range(2):
        hp = slice(64 * h, 64 * (h + 1))
        nc.tensor.matmul(
            ps[:, 512 * h:512 * (h + 1)],
            _kslice(s, kT, h, i), qT[hp, jsl],
            start=True, stop=True)
    ex = s.exp_pool.tile([128, 1024], BF16, tag="exp", name="ex")
    nc.scalar.activation(
        out=ex[:, :], in_=ps[:, :],
        func=mybir.ActivationFunctionType.Exp,
        bias=s.maskb[:, 2 + 16 * b + i:3 + 16 * b + i], scale=1.0)
    return ex


def _emit_pv(nc, s, b, j, i, ex):
    _, _, v_sb = s.proj[b]
    s.py = getattr(s, "py", {})
    if (b, j) not in s.py:
        s.py[(b, j)] = [
            s.py_pool.tile([128, 512], F32, tag="py", name=f"py{h}")
            for h in range(2)]
    py = s.py[(b, j)]
    for h in range(2):
        nc.tensor.matmul(
            py[h][:, :], v_sb[:, i, 128 * h:128 * (h + 1)],
            ex[:, 512 * h:512 * (h + 1)],
            start=(i == 0), stop=(i == SK - 1))


def _gen_tail(nc, s, b, j):
    """Normalize + transpose back + store for one (b, j) block.

    py rows (both heads): 0-63 y values, 64 denom, 65-127 junk/zeros.
    The DVE casts rows 0-95 to bf16 (rows 65-95 transpose into ignored
    columns), one DMA xbar transpose per head flips to [query, feature]
    (query order permuted as q = 4p + t), and the DVE normalizes into a
    [128, 4, 128] f32 block stored with a single DMA whose DRAM access
    pattern undoes the permutation.
    """
    mult = mybir.AluOpType.mult
    py = s.py.pop((b, j))
    ys = []
    for h in range(2):
        y = s.y_pool.tile([128, 512], BF16, tag="y", name=f"y{h}")
        nc.vector.tensor_copy(out=y[0:96, :], in_=py[h][0:96, :])
        ys.append(y)
    yield "t"
    y_nat = s.ynat_pool.tile([128, 8, 96], BF16, tag="ynat", name="ynat")
    for h in range(2):
        nc.sync.dma_start_transpose(
            out=y_nat[:, 4 * h:4 * (h + 1), :], in_=ys[h][0:96, :])
    yield "t"
    ob = s.out_pool.tile([128, 4, DHC], F32, tag="outs", name="ob")
    for t in range(4):
        for h in range(2):
            rc = s.rc_pool.tile([128, 1], F32, tag="rc", name="rc")
            nc.vector.reciprocal(rc[:, :], y_nat[:, 4 * h + t, 64:65])
            nc.vector.tensor_scalar(
                out=ob[:, t, 64 * h:64 * (h + 1)],
                in0=y_nat[:, 4 * h + t, 0:64],
                scalar1=rc[:, :], scalar2=None, op0=mult)
        if s.has_bv:
            nc.vector.tensor_add(ob[:, t, :], ob[:, t, :], s.bv_sb[:, :])
        yield "t"
    pat = "(p t) d -> p t d" if PMAJ else "(t p) d -> p t d"
    dst = s.out[b, 512 * j:512 * (j + 1), :].rearrange(pat, t=4)
    nc.sync.dma_start(out=dst, in_=ob[:, :, :])
    yield "t"


def _emit_body(nc, tc, ctx, aps, has_bv):
    from collections import deque

    s = _setup(nc, tc, ctx, aps, has_bv)
    _x_load_b0(nc, s)
    _warm_act(nc, s)
    # Tile derives dependencies from emission order, so proj work must be
    # emitted before the attention instructions that read it; the drip
    # pacing below keeps the PE queue fed without letting projection
    # matmuls that wait on not-yet-landed xT slices block the queue.
    pend = deque()
    tails = deque()
    gp0 = _gen_proj(nc, s, 0, split=True)
    for tok in gp0:
        if tok == "BOOT":
            break
    pend.append(gp0)

    DONE = object()

    def drip(n):
        for _ in range(n):
            if tails:
                if next(tails[0], DONE) is DONE:
                    tails.popleft()
                continue
            if pend:
                if next(pend[0], DONE) is DONE:
                    pend.popleft()

    slots = [(b, j, i) for b in range(B) for j in range(SQ) for i in range(SK)]
    lag = deque()
    for n, (b, j, i) in enumerate(slots):
        # Next batch's xT chunk dispatches go out inline (sync queue only,
        # no PE instructions) once this batch's own v/y DMA traffic has
        # been emitted; the projection matmuls are dripped only from slot
        # 24 on, by which time the xT transfers have landed -- an earlier
        # emission would block the in-order PE queue on the transfer.
        if b + 1 < B:
            m = 16 * j + i  # slot within batch b
            if m == (12 if b == 0 else 6):
                _x_load(nc, s, b + 1)
            if m == (30 if b == 0 else 24):
                pend.append(_gen_proj(nc, s, b + 1))
        # PV of slot n-3 goes first: its single LDWEIGHTS hides behind the
        # previous slot's matmul stream, whereas after the scores pair's
        # dual LDWEIGHTS both PE weight buffers are busy and the next load
        # would stall exposed.
        if n >= 3:
            bb, jj, ii, exx = lag.popleft()
            _emit_pv(nc, s, bb, jj, ii, exx)
            if ii == SK - 1:
                tails.append(_gen_tail(nc, s, bb, jj))
        ex = _emit_scores_exp(nc, s, b, j, i)
        lag.append((b, j, i, ex))
        drip(5 if n < 24 else 2)
    while lag:
        bb, jj, ii, exx = lag.popleft()
        _emit_pv(nc, s, bb, jj, ii, exx)
        if ii == SK - 1:
            tails.append(_gen_tail(nc, s, bb, jj))
    while tails or pend:
        drip(1)


def _build(has_bv):
    from contextlib import ExitStack

    nc = bacc.Bacc("TRN2", target_bir_lowering=False, debug=False)
    x = nc.dram_tensor("x", [B, S, E], BF16, kind="ExternalInput").ap()
    wq = nc.dram_tensor("wcat", [128, 3, 8, 128], BF16,
                        kind="ExternalInput").ap()
    bq = nc.dram_tensor("consts", [128, 66], F32, kind="ExternalInput").ap()
    bv = nc.dram_tensor("bv", [1, DHC], F32, kind="ExternalInput").ap()
    out = nc.dram_tensor("out", [B, S, DHC], F32, kind="ExternalOutput").ap()
    aps = (x, wq, None, None, bq, None, bv, None, out)
    with tile.TileContext(nc) as tc:
        with ExitStack() as ctx:
            _emit_body(nc, tc, ctx, aps, has_bv)
    nc.compile()
    return nc


_BUILD_CACHE = {}


def _get_built(has_bv):
    if has_bv not in _BUILD_CACHE:
        _BUILD_CACHE[has_bv] = _build(has_bv)
    return _BUILD_CACHE[has_bv]


def kernel(x, mask, Wq, bq, Wk, bk, Wv, bv):
    global LAST_RESULTS
    bf16 = ml_dtypes.bfloat16
    x_bf = np.asarray(x, dtype=np.float32).astype(bf16)
    mask_f = np.asarray(mask).astype(np.float32)
    maskb = (mask_f - 1.0) * (-NEG)  # 0 where mask==1, NEG where mask==0
    if PMAJ:
        # col (b, i=4*jh+t)[p] = bias for key s = 512*jh + 4*p + t
        maskb = np.ascontiguousarray(
            maskb.reshape(B, 4, 128, 4).transpose(2, 0, 1, 3)
            .reshape(128, B, 16)).astype(np.float32)
    else:
        maskb = np.ascontiguousarray(
            maskb.reshape(B, S // 128, 128).transpose(2, 0, 1)
        ).astype(np.float32)

    has_bv = bool(np.any(np.asarray(bv) != 0))
    nc = _get_built(has_bv)

    in_maps = []
    for c in range(NCORES):
        sl = slice(DHC * c, DHC * (c + 1))

        def warr(w):
            w = np.asarray(w, dtype=np.float32)[:, sl].astype(bf16)
            return np.ascontiguousarray(
                w.reshape(8, 128, 128).transpose(1, 0, 2))

        wcat = np.stack([warr(Wq), warr(Wk), warr(Wv)], axis=1)
        consts = np.empty((128, 66), dtype=np.float32)
        consts[:, 0] = np.asarray(bq, dtype=np.float32)[sl] / 8.0
        consts[:, 1] = np.asarray(bk, dtype=np.float32)[sl]
        consts[:, 2:66] = maskb.reshape(128, 64)
        in_maps.append({
            "x": x_bf,
            "wcat": np.ascontiguousarray(wcat),
            "consts": consts,
            "bv": np.ascontiguousarray(
                np.asarray(bv, dtype=np.float32)[sl].reshape(1, DHC)),
        })

    res = run_bass_kernel_spmd(nc, in_maps, core_ids=list(range(NCORES)))
    LAST_RESULTS = res
    return np.concatenate([res.results[c]["out"] for c in range(NCORES)],
                          axis=-1)
